# revision 18
# baseline (speedup 1.0000x reference)
"""Multi-head attention Trainium2 kernel (8 NeuronCores), v2.

Sharding: 8 cores = 4 batches x 2 head-halves (tensor parallel on heads).
Each core computes, for its (batch, 8 heads): q/k/v projections over the FULL
sequence, scores/softmax/ctx for its 4 head-pairs, and a partial output
projection against its half of Wo's rows.  The host sums the two partial
outputs per batch (row-sharded Wo => exact).

Engine plan (per core):
  - Tensor: all matmuls, bf16.  Scores use K=64 tile_position quadrant pairs
    (the two quadrant matmuls execute concurrently on HW).  Projection /
    output-projection matmul groups are spliced between attention tiles on a
    static schedule so the in-order tensor queue always has independent work
    while exp results are pending.
  - Scalar: exact exp (ACT) for 12/16 key-tiles.
  - Vector + Pool(gpsimd): Schraudolph bit-trick exp (bf16 bit pattern via
    int16 affine) for the other 4/16 key-tiles, plus evacuations/normalize.
    Softmax denominators ride along in the ctx matmul as ones-columns.

Bias handling: bq/bk asserted zero; bv/bo corrected exactly on the host
(softmax rows sum to 1).
"""

import os

import numpy as np
import ml_dtypes

B, S, E, H, DH = 4, 2048, 1024, 16, 64
NE = E // 128        # contraction e-tiles
NT = S // 128        # key tiles
NP = 4               # head-pairs per core (8 heads)
NSC = S // 512       # query chunks of 512
HH = 8               # heads per core
NCORES = 8

NBF = 16  # BISECT              # key-tiles 0..NBF-1: bf16 ctx path; rest: fp8 DoubleRow
NTP = (NT - NBF) // 2  # fp8 tile-pairs

# exp engine per key-tile: 'a'=scalar ACT (exact exp, quantized only by the
# output dtype), 'v'=DVE schraudolph (fp8 bit-trick)
EXP_ENG = {t: "a" for t in range(NT)}
for _t in (6, 12):
    EXP_ENG[_t] = "v"

# schraudolph fp8e4: exp(s/8) ~ bitcast_f8e4(int8(round(s*A + B)))
# A folds the 1/8 score scale; DVE f32->i8 rounds to nearest
SCHR_A8 = float(1.0 / np.log(2.0))
SCHR_B8 = 55.54
# bf16 variant (for bf16-path tiles on DVE)
SCHR_A = float(16.0 / np.log(2.0))
SCHR_B = float(127.0 * 128.0 - 5.8 + 0.5)

_cache = {}


def _build():
    import concourse.mybir as mybir
    import concourse.tile as tile
    from concourse import bacc
    from contextlib import ExitStack

    f32 = mybir.dt.float32
    bf16 = mybir.dt.bfloat16
    f8e4 = mybir.dt.float8e4
    i8 = mybir.dt.int8
    i16 = mybir.dt.int16
    EXP = mybir.ActivationFunctionType.Exp
    MULT = mybir.AluOpType.mult
    ADD = mybir.AluOpType.add
    DR = mybir.MatmulPerfMode.DoubleRow

    nc = bacc.Bacc("TRN2", target_bir_lowering=False, debug=False,
                   num_devices=NCORES)

    xT_d = nc.dram_tensor("xT", [E, S], bf16, kind="ExternalInput")
    wq_d = nc.dram_tensor("wq", [E, 512], bf16, kind="ExternalInput")
    wk_d = nc.dram_tensor("wk", [E, 512], bf16, kind="ExternalInput")
    wv_d = nc.dram_tensor("wv", [E, 512], bf16, kind="ExternalInput")
    wo_d = nc.dram_tensor("wo", [512, E], bf16, kind="ExternalInput")
    out_d = nc.dram_tensor("out", [S, E], bf16, kind="ExternalOutput")

    with tile.TileContext(nc) as tc, ExitStack() as top:
        singles = top.enter_context(tc.tile_pool(name="singles", bufs=1))
        sb_kt = top.enter_context(tc.tile_pool(name="sb_kt", bufs=2))
        sb_pt = top.enter_context(tc.tile_pool(name="sb_pt", bufs=5))
        sb_pt8 = top.enter_context(tc.tile_pool(name="sb_pt8", bufs=3))
        sb_nm = top.enter_context(tc.tile_pool(name="sb_nm", bufs=1))
        sb_out = top.enter_context(tc.tile_pool(name="sb_out", bufs=4))
        ps_s = top.enter_context(tc.tile_pool(name="ps_s", bufs=2, space="PSUM"))
        ps_cd = top.enter_context(tc.tile_pool(name="ps_cd", bufs=2, space="PSUM"))
        ps_qk = top.enter_context(tc.tile_pool(name="ps_qk", bufs=2, space="PSUM"))

        qT_sb = singles.tile([128, NP, S], bf16)
        ctx_sb = singles.tile([128, NP, S], bf16)
        v_sb = singles.tile([128, NBF, HH, 128], bf16)
        v8_sb = (singles.tile([128, NTP, 2, HH, 128], f8e4)
                 if NTP else None)
        wv_sb = singles.tile([128, NE, 512], bf16)
        wo_sb = singles.tile([128, NP, E], bf16)
        # per-chunk tiles so matmuls wait on single DMAs, not whole arrays
        xT_t = {(e, h): singles.tile([128, 1024], bf16, name=f"xT{e}_{h}")
                for e in range(NE) for h in range(2)}
        wq_t = {e: singles.tile([128, 512], bf16, name=f"wq{e}")
                for e in range(NE)}
        wk_t = {e: singles.tile([128, 512], bf16, name=f"wk{e}")
                for e in range(NE)}

        def xsl(e, c0, c1):
            h = c0 // 1024
            return xT_t[(e, h)][:, c0 - h * 1024:c1 - h * 1024]

        xT_r = xT_d.rearrange("(eo ei) t -> ei eo t", ei=128)
        wq_r = wq_d.rearrange("(eo ei) h -> ei eo h", ei=128)
        wk_r = wk_d.rearrange("(eo ei) h -> ei eo h", ei=128)
        wv_r = wv_d.rearrange("(eo ei) h -> ei eo h", ei=128)
        wo_r = wo_d.rearrange("(ho hi) e -> hi ho e", hi=128)

        # prologue DMAs on four queues in parallel: wq then wv/wo on pool,
        # wk on vector, xT half0 on sync, xT half1 on scalar
        for e in range(NE):
            nc.gpsimd.dma_start(out=wq_t[e][:], in_=wq_r[:, e, :])
            nc.scalar.dma_start(out=wk_t[e][:], in_=wk_r[:, e, :])
            nc.sync.dma_start(out=xT_t[(e, 0)][:], in_=xT_r[:, e, 0:1024])
        for e in range(NE):
            nc.scalar.dma_start(out=xT_t[(e, 1)][:], in_=xT_r[:, e, 1024:2048])
        for e in range(NE):
            nc.gpsimd.dma_start(out=wv_sb[:, e, :], in_=wv_r[:, e, :])
        for j in range(NP):
            nc.gpsimd.dma_start(out=wo_sb[:, j, :], in_=wo_r[:, j, :])

        # ones columns for the softmax denominators (chunked over key
        # tiles so the big memsets don't head-block prologue evacuations)
        def ones_memset(t0, t1):
            nc.vector.memset(v_sb[:, t0:t1, 0:8:2, 64:128], 1.0)
            nc.vector.memset(v_sb[:, t0:t1, 1:8:2, 0:64], 1.0)

        def ones_memset_f8(p0, p1):
            if not NTP:
                return
            nc.vector.memset(v8_sb[:, p0:p1, :, 0:8:2, 64:128], 1.0)
            nc.vector.memset(v8_sb[:, p0:p1, :, 1:8:2, 0:64], 1.0)

        ones_memset(0, 2)

        kt_tiles = {}

        qk_ps = {}

        def q_group(j, sc, part=2):
            def fn():
                if (j, sc) not in qk_ps:
                    qk_ps[(j, sc)] = ps_qk.tile([128, 512], f32, tag="ps_qk",
                                                name="ps_q")
                ps = qk_ps[(j, sc)]
                es = range(4) if part == 0 else range(4, NE) if part == 1 \
                    else range(NE)
                for e in es:
                    nc.tensor.matmul(
                        ps[:], wq_t[e][:, j * 128:(j + 1) * 128],
                        xsl(e, sc * 512, (sc + 1) * 512),
                        start=(e == 0), stop=(e == NE - 1))
                if part != 0:
                    qk_ps.pop((j, sc))
                    nc.vector.tensor_copy(
                        qT_sb[:, j, sc * 512:(sc + 1) * 512], ps[:])
            return fn

        def k_group(j, tch, part=2):
            def fn():
                if tch == 0 and part != 1:
                    kt_tiles[j] = sb_kt.tile([128, S], bf16, tag="kt", name="kt")
                kt = kt_tiles[j]
                if (j, "k", tch) not in qk_ps:
                    qk_ps[(j, "k", tch)] = ps_qk.tile([128, 512], f32,
                                                      tag="ps_qk", name="ps_k")
                ps = qk_ps[(j, "k", tch)]
                es = range(4) if part == 0 else range(4, NE) if part == 1 \
                    else range(NE)
                for e in es:
                    nc.tensor.matmul(
                        ps[:], wk_t[e][:, j * 128:(j + 1) * 128],
                        xsl(e, tch * 512, (tch + 1) * 512),
                        start=(e == 0), stop=(e == NE - 1))
                if part != 0:
                    qk_ps.pop((j, "k", tch))
                    nc.vector.tensor_copy(kt[:, tch * 512:(tch + 1) * 512], ps[:])
            return fn

        def v_group(t):
            # all 8 heads' V for one key-tile in a single N=512 matmul chain
            def fn():
                ps = ps_qk.tile([128, 512], f32, tag="ps_qk")
                for e in range(NE):
                    nc.tensor.matmul(
                        ps[:], xsl(e, t * 128, (t + 1) * 128),
                        wv_sb[:, e, 0:512],
                        start=(e == 0), stop=(e == NE - 1))
                pv = ps.rearrange("p (j two d) -> p j two d", two=2, d=64)
                if t < NBF:
                    nc.vector.tensor_copy(v_sb[:, t, 0:8:2, 0:64], pv[:, :, 0, :])
                    nc.vector.tensor_copy(v_sb[:, t, 1:8:2, 64:128],
                                          pv[:, :, 1, :])
                else:
                    tp, ko = divmod(t - NBF, 2)
                    nc.vector.tensor_copy(v8_sb[:, tp, ko, 0:8:2, 0:64],
                                          pv[:, :, 0, :])
                    nc.vector.tensor_copy(v8_sb[:, tp, ko, 1:8:2, 64:128],
                                          pv[:, :, 1, :])
            return fn

        def out_group(st, half, scalar_evac=False):
            def fn():
                ps = ps_qk.tile([128, 512], f32, tag="ps_qk")
                for j in range(NP):
                    nc.tensor.matmul(
                        ps[:], ctx_sb[:, j, st * 128:(st + 1) * 128],
                        wo_sb[:, j, half * 512:(half + 1) * 512],
                        start=(j == 0), stop=(j == NP - 1))
                ot = sb_out.tile([128, 512], bf16, tag="out")
                if scalar_evac:
                    nc.scalar.copy(ot[:], ps[:])
                else:
                    nc.vector.tensor_copy(ot[:], ps[:])
                eng = nc.sync if st % 2 == 0 else nc.gpsimd
                eng.dma_start(
                    out=out_d[st * 128:(st + 1) * 128,
                              half * 512:(half + 1) * 512],
                    in_=ot[:])
            return fn

        # ---- static splice schedule: global iter (j*64 + sc*16 + t) -> work
        sched = {}

        def at(g, fn):
            sched.setdefault(g, []).append(fn)

        at(2, q_group(0, 1, 0))
        at(3, q_group(0, 1, 1))
        at(18, q_group(0, 2, 0))
        at(19, q_group(0, 2, 1))
        at(34, q_group(0, 3, 0))
        at(35, q_group(0, 3, 1))
        for t in range(6, NT):
            at(t - 6, v_group(t))
        for j in range(1, NP):
            base = (j - 1) * 64
            for tch in range(4):
                at(base + 4 + 4 * tch, k_group(j, tch, 0))
                at(base + 5 + 4 * tch, k_group(j, tch, 1))
            for sc in (0, 1):
                at(base + 22 + 6 * sc, q_group(j, sc, 0))
                at(base + 23 + 6 * sc, q_group(j, sc, 1))
            for sc in (2, 3):
                at(j * 64 + 2 + 4 * (sc - 2), q_group(j, sc, 0))
                at(j * 64 + 3 + 4 * (sc - 2), q_group(j, sc, 1))
        for sc in range(NSC - 1):
            for i, st in enumerate(range(4 * sc, 4 * sc + 4)):
                at(192 + (sc + 1) * 16 + 1 + 2 * (2 * i), out_group(st, 0))
                at(192 + (sc + 1) * 16 + 2 + 2 * (2 * i + 1), out_group(st, 1))

        def pump(g):
            for fn in sched.pop(g, []):
                fn()

        def ctx_mm(j, t, pt, ps_ca, ps_cb, start, stop):
            nc.tensor.matmul(ps_ca[:], v_sb[:, t, 2 * j, :], pt[:, 0:512],
                             start=start, stop=stop)
            nc.tensor.matmul(ps_cb[:], v_sb[:, t, 2 * j + 1, :],
                             pt[:, 512:1024], start=start, stop=stop)

        def ctx_dr(j, tp, pt8, ps_ca, ps_cb, stop):
            # fp8 DoubleRow: one MM contracts both key-tiles of the pair
            nc.tensor.matmul(ps_ca[:], v8_sb[:, tp, :, 2 * j, :],
                             pt8[:, 0, :, :], start=False, stop=stop,
                             perf_mode=DR)
            nc.tensor.matmul(ps_cb[:], v8_sb[:, tp, :, 2 * j + 1, :],
                             pt8[:, 1, :, :], start=False, stop=stop,
                             perf_mode=DR)

        def normalize(j, sc, ps_ca, ps_cb):
            tA = sb_nm.tile([128, 512], f32, tag="tA")
            tB = sb_nm.tile([128, 512], f32, tag="tB")
            nc.vector.tensor_copy(tA[:], ps_ca[:])
            nc.vector.tensor_copy(tB[:], ps_cb[:])
            # head A: denom replicated at rows 64:128; one row -> partition 0,
            # reciprocal, broadcast back to rows 0:64
            rA = sb_nm.tile([1, 512], f32, tag="rA")
            rbA = sb_nm.tile([64, 512], f32, tag="rbA")
            nc.sync.dma_start(out=rA[0:1, :], in_=tA[64:65, :])
            nc.vector.reciprocal_approx_fast(rA[0:1, :], rA[0:1, :])
            nc.gpsimd.partition_broadcast(rbA[:, :], rA[0:1, :])
            nc.vector.tensor_mul(
                ctx_sb[0:64, j, sc * 512:(sc + 1) * 512], tA[0:64, :], rbA[:, :])
            # head B: denom at row 0 already
            rB = sb_nm.tile([1, 512], f32, tag="rB")
            rbB = sb_nm.tile([128, 512], f32, tag="rbB")
            nc.vector.reciprocal_approx_fast(rB[0:1, :], tB[0:1, :])
            nc.gpsimd.partition_broadcast(rbB[:, :], rB[0:1, :])
            nc.vector.tensor_mul(
                ctx_sb[64:128, j, sc * 512:(sc + 1) * 512], tB[64:128, :],
                rbB[64:128, :])

        # ---- prologue compute
        q_group(0, 0)()
        ones_memset(2, 6)
        for tch in range(4):
            k_group(0, tch)()
            if tch == 1:
                ones_memset(6, NBF)
                ones_memset_f8(0, 2)
        ones_memset_f8(2, NTP)
        for t in range(6):
            v_group(t)()

        # ---- main attention loop
        for j in range(NP):
            kt = None
            for sc in range(NSC):
                ps_ca = ps_cd.tile([128, 512], f32, tag="ps_cd")
                ps_cb = ps_cd.tile([128, 512], f32, tag="ps_cd")
                pts = {}
                pt8s = {}
                for t in range(NT):
                    kt = kt_tiles[j]
                    ps_sc_t = ps_s.tile([128, 1024], f32, tag="ps_s", name="ps_sc_t")
                    nc.tensor.matmul(
                        ps_sc_t[:, 0:512], kt[0:64, t * 128:(t + 1) * 128],
                        qT_sb[0:64, j, sc * 512:(sc + 1) * 512],
                        start=True, stop=True, tile_position=(0, 0))
                    nc.tensor.matmul(
                        ps_sc_t[:, 512:1024], kt[64:128, t * 128:(t + 1) * 128],
                        qT_sb[64:128, j, sc * 512:(sc + 1) * 512],
                        start=True, stop=True, tile_position=(64, 0))
                    if t < NBF:
                        pt = sb_pt.tile([128, 1024], bf16, tag="pt", name="pt")
                        if EXP_ENG[t] == "a":
                            nc.scalar.activation(pt[:], ps_sc_t[:], EXP,
                                                 scale=0.125)
                        else:
                            nc.vector.tensor_scalar(pt.bitcast(i16)[:],
                                                    ps_sc_t[:], SCHR_A,
                                                    SCHR_B, MULT, ADD)
                        pts[t] = pt
                    else:
                        tp, ko = divmod(t - NBF, 2)
                        if ko == 0:
                            pt8s[tp] = sb_pt8.tile([128, 2, 2, 512], f8e4,
                                                   tag="pt8", name="pt8")
                        dst = pt8s[tp][:, :, ko, :]
                        if EXP_ENG[t] == "a":
                            nc.scalar.activation(dst, ps_sc_t[:], EXP,
                                                 scale=0.125)
                        else:
                            nc.vector.tensor_scalar(dst.bitcast(i8),
                                                    ps_sc_t[:], SCHR_A8,
                                                    SCHR_B8, MULT, ADD)
                    pump(j * 64 + sc * 16 + t)
                    if 2 <= t < min(NBF, NT - 2) + 2:
                        ctx_mm(j, t - 2, pts.pop(t - 2), ps_ca, ps_cb,
                               start=(t == 2), stop=False)
                    elif t >= NBF + 3 and (t - NBF - 3) % 2 == 0:
                        tp = (t - NBF - 3) // 2
                        ctx_dr(j, tp, pt8s.pop(tp), ps_ca, ps_cb, stop=False)
                if NTP > 0:
                    ctx_dr(j, NTP - 1, pt8s.pop(NTP - 1), ps_ca, ps_cb,
                           stop=True)
                else:
                    for tl in (NT - 2, NT - 1):
                        ctx_mm(j, tl, pts.pop(tl), ps_ca, ps_cb,
                               start=False, stop=(tl == NT - 1))
                normalize(j, sc, ps_ca, ps_cb)

        # ---- tail: remaining scheduled work, then the last query-chunk's
        # output projection
        for g in sorted(sched.keys()):
            for fn in sched.pop(g, []):
                fn()
        for st in range(12, 16):
            out_group(st, 0, scalar_evac=True)()
            out_group(st, 1, scalar_evac=True)()

    nc.compile()
    return nc


def _prep(xs, Wq, Wk, Wv, Wo):
    bf = ml_dtypes.bfloat16
    xT_b = [np.ascontiguousarray(xs[b].T).astype(bf) for b in range(B)]
    halves = []
    for g in range(2):
        hsl = slice(g * 8, (g + 1) * 8)
        halves.append({
            "wq": np.ascontiguousarray(
                Wq[hsl].transpose(1, 0, 2).reshape(E, 512)).astype(bf),
            "wk": np.ascontiguousarray(
                Wk[hsl].transpose(1, 0, 2).reshape(E, 512)).astype(bf),
            "wv": np.ascontiguousarray(
                Wv[hsl].transpose(1, 0, 2).reshape(E, 512)).astype(bf),
            "wo": np.ascontiguousarray(Wo[g * 512:(g + 1) * 512]).astype(bf),
        })
    in_maps = []
    for c in range(NCORES):
        b, g = divmod(c, 2)
        m = {"xT": xT_b[b]}
        m.update(halves[g])
        in_maps.append(m)
    return in_maps


def kernel(xs, Wq, bq, Wk, bk, Wv, bv, Wo, bo):
    from concourse.bass_utils import run_bass_kernel_spmd

    if "nc" not in _cache:
        _cache["nc"] = _build()
    nc = _cache["nc"]

    xs = np.asarray(xs, dtype=np.float32)
    Wq = np.asarray(Wq, dtype=np.float32)
    Wk = np.asarray(Wk, dtype=np.float32)
    Wv = np.asarray(Wv, dtype=np.float32)
    Wo = np.asarray(Wo, dtype=np.float32)
    bq = np.asarray(bq, dtype=np.float32)
    bk = np.asarray(bk, dtype=np.float32)
    bv = np.asarray(bv, dtype=np.float32)
    bo = np.asarray(bo, dtype=np.float32)
    assert not (np.any(bq) or np.any(bk)), "nonzero bq/bk not supported"

    in_maps = _prep(xs, Wq, Wk, Wv, Wo)

    trace = bool(int(os.environ.get("BASS_KERNEL_TRACE", "0")))
    if trace:
        try:
            import antenv.axon_hooks  # noqa: F401  (registered by the harness)
        except ImportError:
            trace = False
    kw = dict(trace=True, trace_cores=[0]) if trace else {}
    res = run_bass_kernel_spmd(nc, in_maps, core_ids=list(range(NCORES)), **kw)
    if trace and res.exec_time_ns is not None:
        print(f"HW exec time: {res.exec_time_ns} ns")
        if res.instructions_and_trace is not None:
            print("trace:", res.instructions_and_trace[1])

    out = np.empty((B, S, E), dtype=np.float32)
    for b in range(B):
        out[b] = res.results[2 * b]["out"].astype(np.float32)
        out[b] += res.results[2 * b + 1]["out"].astype(np.float32)

    # exact host-side correction for v/output biases (zero in this problem)
    if np.any(bv) or np.any(bo):
        out += bv.reshape(E) @ Wo + bo
    return out



# revision 23
# speedup vs baseline: 1.0441x; 1.0441x over previous
"""Multi-head attention Trainium2 kernel (8 NeuronCores), v4.

Sharding: 8 cores = 4 batches x 2 head-halves (tensor parallel on heads).
Each core computes, for its (batch, 8 heads): q/k/v projections over the FULL
sequence, scores/softmax/ctx for its 4 head-pairs, and a partial output
projection against its half of Wo's rows.  The host sums the two partial
outputs per batch (row-sharded Wo => exact).

Engine plan (per core):
  - Tensor: all matmuls, bf16.  Scores use K=64 tile_position quadrant pairs
    (concurrent on HW).  Ctx matmuls for (j,sc) are DEFERRED: they run as
    back-to-back filler spread through the NEXT (j,sc)'s score loop, so their
    ldweights hide behind in-flight matmuls and exp latency never stalls PE.
  - Scalar: exact exp (ACT) for 14/16 key-tiles.
  - Vector: Schraudolph bf16 exp for 2/16 key-tiles, evacuations, normalize.
    Softmax denominators ride along in the ctx matmul as ones-columns.

Bias handling: bq/bk asserted zero; bv/bo corrected exactly on the host
(softmax rows sum to 1).
"""

import os

import numpy as np
import ml_dtypes

B, S, E, H, DH = 4, 2048, 1024, 16, 64
NE = E // 128        # contraction e-tiles
NT = S // 128        # key tiles
NP = 4               # head-pairs per core (8 heads)
NSC = S // 512       # query chunks of 512
HH = 8               # heads per core
NCORES = 8

NBF = 16             # key-tiles 0..NBF-1: bf16 ctx path; rest: fp8 DoubleRow
NTP = (NT - NBF) // 2  # fp8 tile-pairs

# exp engine per key-tile: 'a'=scalar ACT (exact), 'v'=DVE schraudolph
EXP_ENG = {t: "a" for t in range(NT)}
for _t in (6, 12):
    EXP_ENG[_t] = "v"

# schraudolph bf16: exp(s/8) ~ bitcast_bf16(int16(round(s*A + B)))
SCHR_A = float(16.0 / np.log(2.0))
SCHR_B = float(127.0 * 128.0 - 5.8 + 0.5)
# schraudolph fp8e4 variant (DVE f32->i8 rounds to nearest)
SCHR_A8 = float(1.0 / np.log(2.0))
SCHR_B8 = 55.54

_cache = {}


def _build():
    import concourse.mybir as mybir
    import concourse.tile as tile
    from concourse import bacc
    from contextlib import ExitStack

    f32 = mybir.dt.float32
    bf16 = mybir.dt.bfloat16
    f8e4 = mybir.dt.float8e4
    i8 = mybir.dt.int8
    i16 = mybir.dt.int16
    EXP = mybir.ActivationFunctionType.Exp
    MULT = mybir.AluOpType.mult
    ADD = mybir.AluOpType.add
    DR = mybir.MatmulPerfMode.DoubleRow

    nc = bacc.Bacc("TRN2", target_bir_lowering=False, debug=False,
                   num_devices=NCORES)

    xT_d = nc.dram_tensor("xT", [E, S], bf16, kind="ExternalInput")
    wq_d = nc.dram_tensor("wq", [E, 512], bf16, kind="ExternalInput")
    wk_d = nc.dram_tensor("wk", [E, 512], bf16, kind="ExternalInput")
    wv_d = nc.dram_tensor("wv", [E, 512], bf16, kind="ExternalInput")
    wo_d = nc.dram_tensor("wo", [512, E], bf16, kind="ExternalInput")
    out_d = nc.dram_tensor("out", [S, E], bf16, kind="ExternalOutput")

    with tile.TileContext(nc) as tc, ExitStack() as top:
        singles = top.enter_context(tc.tile_pool(name="singles", bufs=1))
        sb_kt = top.enter_context(tc.tile_pool(name="sb_kt", bufs=2))
        sb_pt = top.enter_context(tc.tile_pool(name="sb_pt", bufs=18))
        sb_nm = top.enter_context(tc.tile_pool(name="sb_nm", bufs=1))
        sb_out = top.enter_context(tc.tile_pool(name="sb_out", bufs=4))
        ps_s = top.enter_context(tc.tile_pool(name="ps_s", bufs=2, space="PSUM"))
        ps_cd = top.enter_context(tc.tile_pool(name="ps_cd", bufs=2, space="PSUM"))
        ps_qk = top.enter_context(tc.tile_pool(name="ps_qk", bufs=2, space="PSUM"))

        qT_sb = singles.tile([128, NP, S], bf16)
        ctx_sb = singles.tile([128, NP, S], bf16)
        v_sb = singles.tile([128, NBF, HH, 128], bf16)
        v8_sb = (singles.tile([128, NTP, 2, HH, 128], f8e4, name="v8_sb")
                 if NTP else None)
        wv_sb = singles.tile([128, NE, 512], bf16)
        wo_sb = singles.tile([128, NP, E], bf16)
        # per-chunk tiles so matmuls wait on single DMAs, not whole arrays
        xT_t = {(e, h): singles.tile([128, 1024], bf16, name=f"xT{e}_{h}")
                for e in range(NE) for h in range(2)}
        wq_t = {e: singles.tile([128, 512], bf16, name=f"wq{e}")
                for e in range(NE)}
        wk_t = {e: singles.tile([128, 512], bf16, name=f"wk{e}")
                for e in range(NE)}

        def xsl(e, c0, c1):
            h = c0 // 1024
            return xT_t[(e, h)][:, c0 - h * 1024:c1 - h * 1024]

        xT_r = xT_d.rearrange("(eo ei) t -> ei eo t", ei=128)
        wq_r = wq_d.rearrange("(eo ei) h -> ei eo h", ei=128)
        wk_r = wk_d.rearrange("(eo ei) h -> ei eo h", ei=128)
        wv_r = wv_d.rearrange("(eo ei) h -> ei eo h", ei=128)
        wo_r = wo_d.rearrange("(ho hi) e -> hi ho e", hi=128)

        # prologue DMAs on three queues in parallel, ordered by first use:
        # sync: xT half0 then wo; scalar: wk then xT half1; gpsimd: wq then wv
        for e in range(NE):
            nc.gpsimd.dma_start(out=wq_t[e][:], in_=wq_r[:, e, :])
            nc.scalar.dma_start(out=wk_t[e][:], in_=wk_r[:, e, :])
            nc.sync.dma_start(out=xT_t[(e, 0)][:], in_=xT_r[:, e, 0:1024])
        for e in range(NE):
            nc.scalar.dma_start(out=xT_t[(e, 1)][:], in_=xT_r[:, e, 1024:2048])
            nc.gpsimd.dma_start(out=wv_sb[:, e, :], in_=wv_r[:, e, :])
        for j in range(NP):
            nc.sync.dma_start(out=wo_sb[:, j, :], in_=wo_r[:, j, :])

        # ones columns for the softmax denominators (chunked over key
        # tiles so the big memsets don't head-block prologue evacuations)
        def ones_memset(t0, t1):
            nc.vector.memset(v_sb[:, t0:t1, 0:8:2, 64:128], 1.0)
            nc.vector.memset(v_sb[:, t0:t1, 1:8:2, 0:64], 1.0)

        def ones_memset_f8(p0, p1):
            if not NTP:
                return
            nc.vector.memset(v8_sb[:, p0:p1, :, 0:8:2, 64:128], 1.0)
            nc.vector.memset(v8_sb[:, p0:p1, :, 1:8:2, 0:64], 1.0)

        ones_memset(0, 2)

        kt_tiles = {}

        qk_ps = {}

        def q_group(j, sc, part=2):
            def fn():
                if (j, sc) not in qk_ps:
                    qk_ps[(j, sc)] = ps_qk.tile([128, 512], f32, tag="ps_qk",
                                                name="ps_q")
                ps = qk_ps[(j, sc)]
                es = range(4) if part == 0 else range(4, NE) if part == 1 \
                    else range(NE)
                for e in es:
                    nc.tensor.matmul(
                        ps[:], wq_t[e][:, j * 128:(j + 1) * 128],
                        xsl(e, sc * 512, (sc + 1) * 512),
                        start=(e == 0), stop=(e == NE - 1))
                if part != 0:
                    qk_ps.pop((j, sc))
                    nc.vector.tensor_copy(
                        qT_sb[:, j, sc * 512:(sc + 1) * 512], ps[:])
            return fn

        def k_group(j, tch, part=2):
            def fn():
                if tch == 0 and part != 1:
                    kt_tiles[j] = sb_kt.tile([128, S], bf16, tag="kt", name="kt")
                kt = kt_tiles[j]
                if (j, "k", tch) not in qk_ps:
                    qk_ps[(j, "k", tch)] = ps_qk.tile([128, 512], f32,
                                                      tag="ps_qk", name="ps_k")
                ps = qk_ps[(j, "k", tch)]
                es = range(4) if part == 0 else range(4, NE) if part == 1 \
                    else range(NE)
                for e in es:
                    nc.tensor.matmul(
                        ps[:], wk_t[e][:, j * 128:(j + 1) * 128],
                        xsl(e, tch * 512, (tch + 1) * 512),
                        start=(e == 0), stop=(e == NE - 1))
                if part != 0:
                    qk_ps.pop((j, "k", tch))
                    nc.vector.tensor_copy(kt[:, tch * 512:(tch + 1) * 512], ps[:])
            return fn

        def v_group(t):
            # all 8 heads' V for one key-tile in a single N=512 matmul chain
            def fn():
                ps = ps_qk.tile([128, 512], f32, tag="ps_qk")
                for e in range(NE):
                    nc.tensor.matmul(
                        ps[:], xsl(e, t * 128, (t + 1) * 128),
                        wv_sb[:, e, 0:512],
                        start=(e == 0), stop=(e == NE - 1))
                pv = ps.rearrange("p (j two d) -> p j two d", two=2, d=64)
                if t < NBF:
                    nc.vector.tensor_copy(v_sb[:, t, 0:8:2, 0:64], pv[:, :, 0, :])
                    nc.vector.tensor_copy(v_sb[:, t, 1:8:2, 64:128],
                                          pv[:, :, 1, :])
                else:
                    tp, ko = divmod(t - NBF, 2)
                    nc.vector.tensor_copy(v8_sb[:, tp, ko, 0:8:2, 0:64],
                                          pv[:, :, 0, :])
                    nc.vector.tensor_copy(v8_sb[:, tp, ko, 1:8:2, 64:128],
                                          pv[:, :, 1, :])
            return fn

        def out_group(st, half, tail_idx=None):
            def fn():
                ps = ps_qk.tile([128, 512], f32, tag="ps_qk")
                for j in range(NP):
                    nc.tensor.matmul(
                        ps[:], ctx_sb[:, j, st * 128:(st + 1) * 128],
                        wo_sb[:, j, half * 512:(half + 1) * 512],
                        start=(j == 0), stop=(j == NP - 1))
                ot = sb_out.tile([128, 512], bf16, tag="out")
                if tail_idx is None:
                    nc.vector.tensor_copy(ot[:], ps[:])
                else:
                    # tail: rotate evacuation engines so the chain parallelizes
                    eng = (nc.scalar, nc.vector)[tail_idx % 2]
                    if tail_idx % 2 == 0:
                        eng.copy(ot[:], ps[:])
                    else:
                        eng.tensor_copy(ot[:], ps[:])
                dma_eng = (nc.sync, nc.gpsimd, nc.scalar)[(st * 2 + half) % 3]
                dma_eng.dma_start(
                    out=out_d[st * 128:(st + 1) * 128,
                              half * 512:(half + 1) * 512],
                    in_=ot[:])
            return fn

        # ---- static splice schedule: global iter (j*64 + sc*16 + t) -> work
        sched = {}

        def at(g, fn):
            sched.setdefault(g, []).append(fn)

        at(2, q_group(0, 1, 0))
        at(3, q_group(0, 1, 1))
        at(18, q_group(0, 2, 0))
        at(19, q_group(0, 2, 1))
        at(34, q_group(0, 3, 0))
        at(35, q_group(0, 3, 1))
        for t in range(6, NT):
            at(t - 6, v_group(t))
        for j in range(1, NP):
            base = (j - 1) * 64
            for tch in range(4):
                at(base + 4 + 4 * tch, k_group(j, tch, 0))
                at(base + 5 + 4 * tch, k_group(j, tch, 1))
            for sc in (0, 1):
                at(base + 22 + 6 * sc, q_group(j, sc, 0))
                at(base + 23 + 6 * sc, q_group(j, sc, 1))
            for sc in (2, 3):
                at(j * 64 + 2 + 4 * (sc - 2), q_group(j, sc, 0))
                at(j * 64 + 3 + 4 * (sc - 2), q_group(j, sc, 1))
        # O-proj for sc: j=3's normalize(sc) lands at iter 192+(sc+1)*16+8
        # (dense burst); splice the 8 groups into the following iters
        for sc in range(NSC - 1):
            for i, st in enumerate(range(4 * sc, 4 * sc + 4)):
                at(192 + (sc + 1) * 16 + 9 + 2 * i, out_group(st, 0))
                at(192 + (sc + 1) * 16 + 10 + 2 * i, out_group(st, 1))

        def pump(g):
            for fn in sched.pop(g, []):
                fn()

        def ctx_mm(j, t, pt, ps_ca, ps_cb, start, stop):
            nc.tensor.matmul(ps_ca[:], v_sb[:, t, 2 * j, :], pt[:, 0:512],
                             start=start, stop=stop)
            nc.tensor.matmul(ps_cb[:], v_sb[:, t, 2 * j + 1, :],
                             pt[:, 512:1024], start=start, stop=stop)

        def normalize(j, sc, ps_ca, ps_cb):
            tA = sb_nm.tile([128, 512], f32, tag="tA")
            tB = sb_nm.tile([128, 512], f32, tag="tB")
            nc.vector.tensor_copy(tA[:], ps_ca[:])
            nc.vector.tensor_copy(tB[:], ps_cb[:])
            # head A: denom replicated at rows 64:128; one row -> partition 0,
            # reciprocal, broadcast back to rows 0:64
            rA = sb_nm.tile([1, 512], f32, tag="rA")
            rbA = sb_nm.tile([64, 512], f32, tag="rbA")
            nc.sync.dma_start(out=rA[0:1, :], in_=tA[64:65, :])
            nc.vector.reciprocal_approx_fast(rA[0:1, :], rA[0:1, :])
            nc.gpsimd.partition_broadcast(rbA[:, :], rA[0:1, :])
            nc.vector.tensor_mul(
                ctx_sb[0:64, j, sc * 512:(sc + 1) * 512], tA[0:64, :], rbA[:, :])
            # head B: denom at row 0 already
            rB = sb_nm.tile([1, 512], f32, tag="rB")
            rbB = sb_nm.tile([128, 512], f32, tag="rbB")
            nc.vector.reciprocal_approx_fast(rB[0:1, :], tB[0:1, :])
            nc.gpsimd.partition_broadcast(rbB[:, :], rB[0:1, :])
            nc.vector.tensor_mul(
                ctx_sb[64:128, j, sc * 512:(sc + 1) * 512], tB[64:128, :],
                rbB[64:128, :])

        # ---- prologue compute
        q_group(0, 0)()
        ones_memset(2, 6)
        for tch in range(4):
            k_group(0, tch)()
            if tch == 1:
                ones_memset(6, 11)
        ones_memset(11, NBF)
        ones_memset_f8(0, NTP)
        for t in range(6):
            v_group(t)()

        # ---- main attention loop with deferred ctx
        # prev = (j, sc, pts) whose ctx/normalize runs inside the current loop
        prev = None
        for j in range(NP):
            for sc in range(NSC):
                last = (j == NP - 1 and sc == NSC - 1)
                dense = (j == NP - 1)
                if last:
                    # inline ctx accumulators live in ps_qk (free by now)
                    ca_l = ps_qk.tile([128, 512], f32, tag="ps_qk", name="ca_l")
                    cb_l = ps_qk.tile([128, 512], f32, tag="ps_qk", name="cb_l")
                if prev is not None:
                    pj, psc, ppts = prev
                    pca = ps_cd.tile([128, 512], f32, tag="ps_cd")
                    pcb = ps_cd.tile([128, 512], f32, tag="ps_cd")
                pts = {}
                for t in range(NT):
                    kt = kt_tiles[j]
                    ps_sc_t = ps_s.tile([128, 1024], f32, tag="ps_s",
                                        name="ps_sc_t")
                    nc.tensor.matmul(
                        ps_sc_t[:, 0:512], kt[0:64, t * 128:(t + 1) * 128],
                        qT_sb[0:64, j, sc * 512:(sc + 1) * 512],
                        start=True, stop=True, tile_position=(0, 0))
                    nc.tensor.matmul(
                        ps_sc_t[:, 512:1024], kt[64:128, t * 128:(t + 1) * 128],
                        qT_sb[64:128, j, sc * 512:(sc + 1) * 512],
                        start=True, stop=True, tile_position=(64, 0))
                    pt = sb_pt.tile([128, 1024], bf16, tag="pt", name="pt")
                    if EXP_ENG[t] == "a":
                        nc.scalar.activation(pt[:], ps_sc_t[:], EXP,
                                             scale=0.125)
                    else:
                        nc.vector.tensor_scalar(pt.bitcast(i16)[:], ps_sc_t[:],
                                                SCHR_A, SCHR_B, MULT, ADD)
                    pts[t] = pt
                    pump(j * 64 + sc * 16 + t)
                    # deferred ctx burst of prev (j,sc)
                    if prev is not None:
                        if dense:
                            for tt in (2 * t, 2 * t + 1):
                                if tt < NT:
                                    ctx_mm(pj, tt, ppts.pop(tt), pca, pcb,
                                           start=(tt == 0), stop=(tt == NT - 1))
                            if t == 8:
                                normalize(pj, psc, pca, pcb)
                        else:
                            ctx_mm(pj, t, ppts.pop(t), pca, pcb,
                                   start=(t == 0), stop=(t == NT - 1))
                    # inline pipelined ctx for the very last (j,sc)
                    if last and t >= 2:
                        ctx_mm(j, t - 2, pts.pop(t - 2), ca_l, cb_l,
                               start=(t == 2), stop=False)
                if prev is not None and not dense:
                    normalize(pj, psc, pca, pcb)
                if last:
                    for tl in (NT - 2, NT - 1):
                        ctx_mm(j, tl, pts.pop(tl), ca_l, cb_l,
                               start=False, stop=(tl == NT - 1))
                    normalize(j, sc, ca_l, cb_l)
                    prev = None
                else:
                    prev = (j, sc, pts)

        # ---- tail: remaining scheduled work, then the last query-chunk's
        # output projection
        for g in sorted(sched.keys()):
            for fn in sched.pop(g, []):
                fn()
        for i, st in enumerate(range(12, 16)):
            out_group(st, 0, tail_idx=2 * i)()
            out_group(st, 1, tail_idx=2 * i + 1)()

    nc.compile()
    return nc


def _prep(xs, Wq, Wk, Wv, Wo):
    bf = ml_dtypes.bfloat16
    xT_b = [np.ascontiguousarray(xs[b].T).astype(bf) for b in range(B)]
    halves = []
    for g in range(2):
        hsl = slice(g * 8, (g + 1) * 8)
        halves.append({
            "wq": np.ascontiguousarray(
                Wq[hsl].transpose(1, 0, 2).reshape(E, 512)).astype(bf),
            "wk": np.ascontiguousarray(
                Wk[hsl].transpose(1, 0, 2).reshape(E, 512)).astype(bf),
            "wv": np.ascontiguousarray(
                Wv[hsl].transpose(1, 0, 2).reshape(E, 512)).astype(bf),
            "wo": np.ascontiguousarray(Wo[g * 512:(g + 1) * 512]).astype(bf),
        })
    in_maps = []
    for c in range(NCORES):
        b, g = divmod(c, 2)
        m = {"xT": xT_b[b]}
        m.update(halves[g])
        in_maps.append(m)
    return in_maps


def kernel(xs, Wq, bq, Wk, bk, Wv, bv, Wo, bo):
    from concourse.bass_utils import run_bass_kernel_spmd

    if "nc" not in _cache:
        _cache["nc"] = _build()
    nc = _cache["nc"]

    xs = np.asarray(xs, dtype=np.float32)
    Wq = np.asarray(Wq, dtype=np.float32)
    Wk = np.asarray(Wk, dtype=np.float32)
    Wv = np.asarray(Wv, dtype=np.float32)
    Wo = np.asarray(Wo, dtype=np.float32)
    bq = np.asarray(bq, dtype=np.float32)
    bk = np.asarray(bk, dtype=np.float32)
    bv = np.asarray(bv, dtype=np.float32)
    bo = np.asarray(bo, dtype=np.float32)
    assert not (np.any(bq) or np.any(bk)), "nonzero bq/bk not supported"

    in_maps = _prep(xs, Wq, Wk, Wv, Wo)

    trace = bool(int(os.environ.get("BASS_KERNEL_TRACE", "0")))
    if trace:
        try:
            import antenv.axon_hooks  # noqa: F401  (registered by the harness)
        except ImportError:
            trace = False
    kw = dict(trace=True, trace_cores=[0]) if trace else {}
    res = run_bass_kernel_spmd(nc, in_maps, core_ids=list(range(NCORES)), **kw)
    if trace and res.exec_time_ns is not None:
        print(f"HW exec time: {res.exec_time_ns} ns")
        if res.instructions_and_trace is not None:
            print("trace:", res.instructions_and_trace[1])

    out = np.empty((B, S, E), dtype=np.float32)
    for b in range(B):
        out[b] = res.results[2 * b]["out"].astype(np.float32)
        out[b] += res.results[2 * b + 1]["out"].astype(np.float32)

    # exact host-side correction for v/output biases (zero in this problem)
    if np.any(bv) or np.any(bo):
        out += bv.reshape(E) @ Wo + bo
    return out


# revision 30
# speedup vs baseline: 1.0579x; 1.0132x over previous
"""Multi-head attention Trainium2 kernel (8 NeuronCores), v4.

Sharding: 8 cores = 4 batches x 2 head-halves (tensor parallel on heads).
Each core computes, for its (batch, 8 heads): q/k/v projections over the FULL
sequence, scores/softmax/ctx for its 4 head-pairs, and a partial output
projection against its half of Wo's rows.  The host sums the two partial
outputs per batch (row-sharded Wo => exact).

Engine plan (per core):
  - Tensor: all matmuls, bf16.  Scores use K=64 tile_position quadrant pairs
    (concurrent on HW).  Ctx matmuls for (j,sc) are DEFERRED: they run as
    back-to-back filler spread through the NEXT (j,sc)'s score loop, so their
    ldweights hide behind in-flight matmuls and exp latency never stalls PE.
  - Scalar: exact exp (ACT) for 14/16 key-tiles.
  - Vector: Schraudolph bf16 exp for 2/16 key-tiles, evacuations, normalize.
    Softmax denominators ride along in the ctx matmul as ones-columns.

Bias handling: bq/bk asserted zero; bv/bo corrected exactly on the host
(softmax rows sum to 1).
"""

import os

import numpy as np
import ml_dtypes

B, S, E, H, DH = 4, 2048, 1024, 16, 64
NE = E // 128        # contraction e-tiles
NT = S // 128        # key tiles
NP = 4               # head-pairs per core (8 heads)
NSC = S // 512       # query chunks of 512
HH = 8               # heads per core
NCORES = 8

NBF = 12             # key-tiles 0..NBF-1: bf16 ctx path; rest: fp8 DoubleRow
NTP = (NT - NBF) // 2  # fp8 tile-pairs

# exp engine per key-tile: 'a'=scalar ACT (exact), 'v'=DVE schraudolph
EXP_ENG = {t: "a" for t in range(NT)}
for _t in (14, 15):
    EXP_ENG[_t] = "v"

# schraudolph bf16: exp(s/8) ~ bitcast_bf16(int16(round(s*A + B)))
SCHR_A = float(16.0 / np.log(2.0))
SCHR_B = float(127.0 * 128.0 - 5.8 + 0.5)
# schraudolph fp8e4 variant (DVE f32->i8 rounds to nearest)
SCHR_A8 = float(1.0 / np.log(2.0))
SCHR_B8 = 55.54

_cache = {}


def _build():
    import concourse.mybir as mybir
    import concourse.tile as tile
    from concourse import bacc
    from contextlib import ExitStack

    f32 = mybir.dt.float32
    bf16 = mybir.dt.bfloat16
    f8e4 = mybir.dt.float8e4
    i8 = mybir.dt.int8
    i16 = mybir.dt.int16
    EXP = mybir.ActivationFunctionType.Exp
    MULT = mybir.AluOpType.mult
    ADD = mybir.AluOpType.add
    DR = mybir.MatmulPerfMode.DoubleRow

    nc = bacc.Bacc("TRN2", target_bir_lowering=False, debug=False,
                   num_devices=NCORES)

    xT_d = nc.dram_tensor("xT", [E, S], bf16, kind="ExternalInput")
    wq_d = nc.dram_tensor("wq", [E, 512], bf16, kind="ExternalInput")
    wk_d = nc.dram_tensor("wk", [E, 512], bf16, kind="ExternalInput")
    wv_d = nc.dram_tensor("wv", [E, 512], bf16, kind="ExternalInput")
    wo_d = nc.dram_tensor("wo", [512, E], bf16, kind="ExternalInput")
    out_d = nc.dram_tensor("out", [S, E], bf16, kind="ExternalOutput")

    with tile.TileContext(nc) as tc, ExitStack() as top:
        singles = top.enter_context(tc.tile_pool(name="singles", bufs=1))
        sb_kt = top.enter_context(tc.tile_pool(name="sb_kt", bufs=2))
        sb_pt = top.enter_context(tc.tile_pool(name="sb_pt", bufs=15))
        sb_pt8 = top.enter_context(tc.tile_pool(name="sb_pt8", bufs=5))
        sb_nm = top.enter_context(tc.tile_pool(name="sb_nm", bufs=1))
        sb_out = top.enter_context(tc.tile_pool(name="sb_out", bufs=4))
        ps_s = top.enter_context(tc.tile_pool(name="ps_s", bufs=2, space="PSUM"))
        ps_cd = top.enter_context(tc.tile_pool(name="ps_cd", bufs=2, space="PSUM"))
        ps_qk = top.enter_context(tc.tile_pool(name="ps_qk", bufs=2, space="PSUM"))

        qT_sb = singles.tile([128, NP, S], bf16)
        ctx_sb = singles.tile([128, NP, S], bf16)
        v_sb = singles.tile([128, NBF, HH, 128], bf16)
        v8_sb = (singles.tile([128, NTP, 2, HH, 128], f8e4, name="v8_sb")
                 if NTP else None)
        wv_sb = singles.tile([128, NE, 512], bf16)
        wo_sb = singles.tile([128, NP, E], bf16)
        # per-chunk tiles so matmuls wait on single DMAs, not whole arrays
        xT_t = {(e, h): singles.tile([128, 1024], bf16, name=f"xT{e}_{h}")
                for e in range(NE) for h in range(2)}
        wq_t = {e: singles.tile([128, 512], bf16, name=f"wq{e}")
                for e in range(NE)}
        wk_t = {e: singles.tile([128, 512], bf16, name=f"wk{e}")
                for e in range(NE)}

        def xsl(e, c0, c1):
            h = c0 // 1024
            return xT_t[(e, h)][:, c0 - h * 1024:c1 - h * 1024]

        xT_r = xT_d.rearrange("(eo ei) t -> ei eo t", ei=128)
        wq_r = wq_d.rearrange("(eo ei) h -> ei eo h", ei=128)
        wk_r = wk_d.rearrange("(eo ei) h -> ei eo h", ei=128)
        wv_r = wv_d.rearrange("(eo ei) h -> ei eo h", ei=128)
        wo_r = wo_d.rearrange("(ho hi) e -> hi ho e", hi=128)

        # prologue DMAs on three queues in parallel, ordered by first use:
        # sync: xT half0 then wo; scalar: wk then xT half1; gpsimd: wq then wv
        for e in range(NE):
            nc.gpsimd.dma_start(out=wq_t[e][:], in_=wq_r[:, e, :])
            nc.scalar.dma_start(out=wk_t[e][:], in_=wk_r[:, e, :])
            nc.sync.dma_start(out=xT_t[(e, 0)][:], in_=xT_r[:, e, 0:1024])
        for e in range(NE):
            nc.scalar.dma_start(out=xT_t[(e, 1)][:], in_=xT_r[:, e, 1024:2048])
            nc.gpsimd.dma_start(out=wv_sb[:, e, :], in_=wv_r[:, e, :])
        for j in range(NP):
            nc.sync.dma_start(out=wo_sb[:, j, :], in_=wo_r[:, j, :])

        # ones columns for the softmax denominators (chunked over key
        # tiles so the big memsets don't head-block prologue evacuations)
        def ones_memset(t0, t1):
            nc.vector.memset(v_sb[:, t0:t1, 0:8:2, 64:128], 1.0)
            nc.vector.memset(v_sb[:, t0:t1, 1:8:2, 0:64], 1.0)

        def ones_memset_f8(p0, p1):
            if not NTP:
                return
            nc.vector.memset(v8_sb[:, p0:p1, :, 0:8:2, 64:128], 1.0)
            nc.vector.memset(v8_sb[:, p0:p1, :, 1:8:2, 0:64], 1.0)

        ones_memset(0, 2)

        kt_tiles = {}

        qk_ps = {}

        def q_group(j, sc, part=2):
            def fn():
                if (j, sc) not in qk_ps:
                    qk_ps[(j, sc)] = ps_qk.tile([128, 512], f32, tag="ps_qk",
                                                name="ps_q")
                ps = qk_ps[(j, sc)]
                es = range(4) if part == 0 else range(4, NE) if part == 1 \
                    else range(NE)
                for e in es:
                    nc.tensor.matmul(
                        ps[:], wq_t[e][:, j * 128:(j + 1) * 128],
                        xsl(e, sc * 512, (sc + 1) * 512),
                        start=(e == 0), stop=(e == NE - 1))
                if part != 0:
                    qk_ps.pop((j, sc))
                    nc.vector.tensor_copy(
                        qT_sb[:, j, sc * 512:(sc + 1) * 512], ps[:])
            return fn

        def k_group(j, tch, part=2):
            def fn():
                if tch == 0 and part != 1:
                    kt_tiles[j] = sb_kt.tile([128, S], bf16, tag="kt", name="kt")
                kt = kt_tiles[j]
                if (j, "k", tch) not in qk_ps:
                    qk_ps[(j, "k", tch)] = ps_qk.tile([128, 512], f32,
                                                      tag="ps_qk", name="ps_k")
                ps = qk_ps[(j, "k", tch)]
                es = range(4) if part == 0 else range(4, NE) if part == 1 \
                    else range(NE)
                for e in es:
                    nc.tensor.matmul(
                        ps[:], wk_t[e][:, j * 128:(j + 1) * 128],
                        xsl(e, tch * 512, (tch + 1) * 512),
                        start=(e == 0), stop=(e == NE - 1))
                if part != 0:
                    qk_ps.pop((j, "k", tch))
                    nc.vector.tensor_copy(kt[:, tch * 512:(tch + 1) * 512], ps[:])
            return fn

        def v_group(t):
            # all 8 heads' V for one key-tile in a single N=512 matmul chain
            def fn():
                ps = ps_qk.tile([128, 512], f32, tag="ps_qk")
                for e in range(NE):
                    nc.tensor.matmul(
                        ps[:], xsl(e, t * 128, (t + 1) * 128),
                        wv_sb[:, e, 0:512],
                        start=(e == 0), stop=(e == NE - 1))
                pv = ps.rearrange("p (j two d) -> p j two d", two=2, d=64)
                if t < NBF:
                    nc.vector.tensor_copy(v_sb[:, t, 0:8:2, 0:64], pv[:, :, 0, :])
                    nc.vector.tensor_copy(v_sb[:, t, 1:8:2, 64:128],
                                          pv[:, :, 1, :])
                else:
                    tp, ko = divmod(t - NBF, 2)
                    nc.vector.tensor_copy(v8_sb[:, tp, ko, 0:8:2, 0:64],
                                          pv[:, :, 0, :])
                    nc.vector.tensor_copy(v8_sb[:, tp, ko, 1:8:2, 64:128],
                                          pv[:, :, 1, :])
            return fn

        def out_group(st, half, tail_idx=None):
            def fn():
                ps = ps_qk.tile([128, 512], f32, tag="ps_qk")
                for j in range(NP):
                    nc.tensor.matmul(
                        ps[:], ctx_sb[:, j, st * 128:(st + 1) * 128],
                        wo_sb[:, j, half * 512:(half + 1) * 512],
                        start=(j == 0), stop=(j == NP - 1))
                ot = sb_out.tile([128, 512], bf16, tag="out")
                if tail_idx is None:
                    nc.vector.tensor_copy(ot[:], ps[:])
                else:
                    # tail: rotate evacuation engines so the chain parallelizes
                    eng = (nc.scalar, nc.vector)[tail_idx % 2]
                    if tail_idx % 2 == 0:
                        eng.copy(ot[:], ps[:])
                    else:
                        eng.tensor_copy(ot[:], ps[:])
                dma_eng = (nc.sync, nc.gpsimd, nc.scalar)[(st * 2 + half) % 3]
                dma_eng.dma_start(
                    out=out_d[st * 128:(st + 1) * 128,
                              half * 512:(half + 1) * 512],
                    in_=ot[:])
            return fn

        # ---- static splice schedule: global iter (j*64 + sc*16 + t) -> work
        sched = {}

        def at(g, fn):
            sched.setdefault(g, []).append(fn)

        at(2, q_group(0, 1, 0))
        at(3, q_group(0, 1, 1))
        at(18, q_group(0, 2, 0))
        at(19, q_group(0, 2, 1))
        at(34, q_group(0, 3, 0))
        at(35, q_group(0, 3, 1))
        for t in range(6, NT):
            at(t - 6, v_group(t))
        for j in range(1, NP):
            base = (j - 1) * 64
            for tch in range(4):
                at(base + 4 + 4 * tch, k_group(j, tch, 0))
                at(base + 5 + 4 * tch, k_group(j, tch, 1))
            for sc in (0, 1):
                at(base + 22 + 6 * sc, q_group(j, sc, 0))
                at(base + 23 + 6 * sc, q_group(j, sc, 1))
            for sc in (2, 3):
                at(j * 64 + 2 + 4 * (sc - 2), q_group(j, sc, 0))
                at(j * 64 + 3 + 4 * (sc - 2), q_group(j, sc, 1))
        # O-proj for sc: j=3's normalize(sc) lands at iter 192+(sc+1)*16+8
        # (dense burst); splice the 8 groups into the following iters
        for sc in range(NSC - 1):
            for i, st in enumerate(range(4 * sc, 4 * sc + 4)):
                at(192 + (sc + 1) * 16 + 9 + 2 * i, out_group(st, 0))
                at(192 + (sc + 1) * 16 + 10 + 2 * i, out_group(st, 1))

        def pump(g):
            for fn in sched.pop(g, []):
                fn()

        def ctx_mm(j, t, pt, ps_ca, ps_cb, start, stop):
            nc.tensor.matmul(ps_ca[:], v_sb[:, t, 2 * j, :], pt[:, 0:512],
                             start=start, stop=stop)
            nc.tensor.matmul(ps_cb[:], v_sb[:, t, 2 * j + 1, :],
                             pt[:, 512:1024], start=start, stop=stop)

        def ctx_dr(j, tp, pt8, ps_ca, ps_cb, stop):
            # fp8 DoubleRow: one MM contracts both key-tiles of the pair
            nc.tensor.matmul(ps_ca[:], v8_sb[:, tp, :, 2 * j, :],
                             pt8[:, 0, :, :], start=False, stop=stop,
                             perf_mode=DR)
            nc.tensor.matmul(ps_cb[:], v8_sb[:, tp, :, 2 * j + 1, :],
                             pt8[:, 1, :, :], start=False, stop=stop,
                             perf_mode=DR)

        def normalize(j, sc, ps_ca, ps_cb, c0=0, c1=512):
            w = c1 - c0
            tA = sb_nm.tile([128, 512], f32, tag="tA")
            tB = sb_nm.tile([128, 512], f32, tag="tB")
            nc.vector.tensor_copy(tA[:, c0:c1], ps_ca[:, c0:c1])
            nc.vector.tensor_copy(tB[:, c0:c1], ps_cb[:, c0:c1])
            # head A: denom replicated at rows 64:128; one row -> partition 0,
            # reciprocal, broadcast back to rows 0:64
            rA = sb_nm.tile([1, 512], f32, tag="rA")
            rbA = sb_nm.tile([64, 512], f32, tag="rbA")
            nc.sync.dma_start(out=rA[0:1, c0:c1], in_=tA[64:65, c0:c1])
            nc.vector.reciprocal_approx_fast(rA[0:1, c0:c1], rA[0:1, c0:c1])
            nc.gpsimd.partition_broadcast(rbA[:, c0:c1], rA[0:1, c0:c1])
            nc.vector.tensor_mul(
                ctx_sb[0:64, j, sc * 512 + c0:sc * 512 + c1], tA[0:64, c0:c1],
                rbA[:, c0:c1])
            # head B: denom at row 0 already
            rB = sb_nm.tile([1, 512], f32, tag="rB")
            rbB = sb_nm.tile([128, 512], f32, tag="rbB")
            nc.vector.reciprocal_approx_fast(rB[0:1, c0:c1], tB[0:1, c0:c1])
            nc.gpsimd.partition_broadcast(rbB[:, c0:c1], rB[0:1, c0:c1])
            nc.vector.tensor_mul(
                ctx_sb[64:128, j, sc * 512 + c0:sc * 512 + c1],
                tB[64:128, c0:c1], rbB[64:128, c0:c1])

        # ---- prologue compute, ordered to match DMA arrival: xT half1 (for
        # k 2/3) lands last on the scalar queue, so v-groups run before them
        q_group(0, 0)()
        ones_memset(2, 6)
        k_group(0, 0)()
        k_group(0, 1)()
        ones_memset(6, 11)
        for t in range(6):
            v_group(t)()
            if t == 2:
                ones_memset(11, NBF)
        k_group(0, 2)()
        k_group(0, 3)()
        ones_memset_f8(0, NTP)

        # ---- main attention loop with deferred ctx
        # prev = (j, sc, pts, pt8s) whose ctx/normalize runs in the current
        # loop as back-to-back filler (ldweights hide behind in-flight MMs)
        prev = None
        for j in range(NP):
            for sc in range(NSC):
                last = (j == NP - 1 and sc == NSC - 1)
                dense = (j == NP - 1)
                if last:
                    # inline ctx accumulators live in ps_qk (free by now)
                    ca_l = ps_qk.tile([128, 512], f32, tag="ps_qk", name="ca_l")
                    cb_l = ps_qk.tile([128, 512], f32, tag="ps_qk", name="cb_l")
                if prev is not None:
                    pj, psc, ppts, ppt8s = prev
                    pca = ps_cd.tile([128, 512], f32, tag="ps_cd")
                    pcb = ps_cd.tile([128, 512], f32, tag="ps_cd")
                pts = {}
                pt8s = {}
                for t in range(NT):
                    kt = kt_tiles[j]
                    ps_sc_t = ps_s.tile([128, 1024], f32, tag="ps_s",
                                        name="ps_sc_t")
                    nc.tensor.matmul(
                        ps_sc_t[:, 0:512], kt[0:64, t * 128:(t + 1) * 128],
                        qT_sb[0:64, j, sc * 512:(sc + 1) * 512],
                        start=True, stop=True, tile_position=(0, 0))
                    nc.tensor.matmul(
                        ps_sc_t[:, 512:1024], kt[64:128, t * 128:(t + 1) * 128],
                        qT_sb[64:128, j, sc * 512:(sc + 1) * 512],
                        start=True, stop=True, tile_position=(64, 0))
                    if t < NBF:
                        pt = sb_pt.tile([128, 1024], bf16, tag="pt", name="pt")
                        if EXP_ENG[t] == "a":
                            nc.scalar.activation(pt[:], ps_sc_t[:], EXP,
                                                 scale=0.125)
                        else:
                            nc.vector.tensor_scalar(pt.bitcast(i16)[:],
                                                    ps_sc_t[:], SCHR_A,
                                                    SCHR_B, MULT, ADD)
                        pts[t] = pt
                    else:
                        tp, ko = divmod(t - NBF, 2)
                        if ko == 0:
                            pt8s[tp] = sb_pt8.tile([128, 2, 2, 512], f8e4,
                                                   tag="pt8", name="pt8")
                        dst = pt8s[tp][:, :, ko, :]
                        if EXP_ENG[t] == "a":
                            nc.scalar.activation(dst, ps_sc_t[:], EXP,
                                                 scale=0.125)
                        else:
                            nc.vector.tensor_scalar(dst.bitcast(i8),
                                                    ps_sc_t[:], SCHR_A8,
                                                    SCHR_B8, MULT, ADD)
                    pump(j * 64 + sc * 16 + t)
                    # deferred ctx burst of prev (j,sc): bf16 tiles first,
                    # then the fp8 DoubleRow pairs (back-to-back, ldw hidden)
                    if prev is not None:
                        if dense:
                            for tt in (2 * t, 2 * t + 1):
                                if tt < NBF:
                                    ctx_mm(pj, tt, ppts.pop(tt), pca, pcb,
                                           start=(tt == 0), stop=False)
                                elif tt < NT and tt % 2 == 0:
                                    tp = (tt - NBF) // 2
                                    ctx_dr(pj, tp, ppt8s.pop(tp), pca, pcb,
                                           stop=(tp == NTP - 1))
                            if t == 8:
                                normalize(pj, psc, pca, pcb)
                        else:
                            if t < NBF:
                                ctx_mm(pj, t, ppts.pop(t), pca, pcb,
                                       start=(t == 0), stop=False)
                            elif (t - NBF) % 2 == 0:
                                tp = (t - NBF) // 2
                                ctx_dr(pj, tp, ppt8s.pop(tp), pca, pcb,
                                       stop=(tp == NTP - 1))
                    # inline pipelined ctx for the very last (j,sc)
                    if last and 2 <= t < NBF + 2:
                        ctx_mm(j, t - 2, pts.pop(t - 2), ca_l, cb_l,
                               start=(t == 2), stop=False)
                if prev is not None and not dense:
                    normalize(pj, psc, pca, pcb)
                if last:
                    # inline path covered all bf16 tiles; finish with the
                    # fp8 pairs
                    for tp in range(NTP):
                        ctx_dr(j, tp, pt8s.pop(tp), ca_l, cb_l,
                               stop=(tp == NTP - 1))
                    prev = None
                else:
                    prev = (j, sc, pts, pt8s)

        # ---- tail: remaining scheduled work, the last normalize (split by
        # query half so the first output projections start earlier)
        for g in sorted(sched.keys()):
            for fn in sched.pop(g, []):
                fn()
        j, sc = NP - 1, NSC - 1
        normalize(j, sc, ca_l, cb_l, 0, 256)
        out_group(12, 0, tail_idx=0)()
        out_group(12, 1, tail_idx=1)()
        out_group(13, 0, tail_idx=2)()
        normalize(j, sc, ca_l, cb_l, 256, 512)
        out_group(13, 1, tail_idx=3)()
        for i, st in enumerate(range(14, 16)):
            out_group(st, 0, tail_idx=2 * i + 4)()
            out_group(st, 1, tail_idx=2 * i + 5)()

    nc.compile()
    return nc


def _prep(xs, Wq, Wk, Wv, Wo):
    bf = ml_dtypes.bfloat16
    xT_b = [np.ascontiguousarray(xs[b].T).astype(bf) for b in range(B)]
    halves = []
    for g in range(2):
        hsl = slice(g * 8, (g + 1) * 8)
        halves.append({
            "wq": np.ascontiguousarray(
                Wq[hsl].transpose(1, 0, 2).reshape(E, 512)).astype(bf),
            "wk": np.ascontiguousarray(
                Wk[hsl].transpose(1, 0, 2).reshape(E, 512)).astype(bf),
            "wv": np.ascontiguousarray(
                Wv[hsl].transpose(1, 0, 2).reshape(E, 512)).astype(bf),
            "wo": np.ascontiguousarray(Wo[g * 512:(g + 1) * 512]).astype(bf),
        })
    in_maps = []
    for c in range(NCORES):
        b, g = divmod(c, 2)
        m = {"xT": xT_b[b]}
        m.update(halves[g])
        in_maps.append(m)
    return in_maps


def kernel(xs, Wq, bq, Wk, bk, Wv, bv, Wo, bo):
    from concourse.bass_utils import run_bass_kernel_spmd

    if "nc" not in _cache:
        _cache["nc"] = _build()
    nc = _cache["nc"]

    xs = np.asarray(xs, dtype=np.float32)
    Wq = np.asarray(Wq, dtype=np.float32)
    Wk = np.asarray(Wk, dtype=np.float32)
    Wv = np.asarray(Wv, dtype=np.float32)
    Wo = np.asarray(Wo, dtype=np.float32)
    bq = np.asarray(bq, dtype=np.float32)
    bk = np.asarray(bk, dtype=np.float32)
    bv = np.asarray(bv, dtype=np.float32)
    bo = np.asarray(bo, dtype=np.float32)
    assert not (np.any(bq) or np.any(bk)), "nonzero bq/bk not supported"

    in_maps = _prep(xs, Wq, Wk, Wv, Wo)

    trace = bool(int(os.environ.get("BASS_KERNEL_TRACE", "0")))
    if trace:
        try:
            import antenv.axon_hooks  # noqa: F401  (registered by the harness)
        except ImportError:
            trace = False
    kw = dict(trace=True, trace_cores=[0]) if trace else {}
    res = run_bass_kernel_spmd(nc, in_maps, core_ids=list(range(NCORES)), **kw)
    if trace and res.exec_time_ns is not None:
        print(f"HW exec time: {res.exec_time_ns} ns")
        if res.instructions_and_trace is not None:
            print("trace:", res.instructions_and_trace[1])

    out = np.empty((B, S, E), dtype=np.float32)
    for b in range(B):
        out[b] = res.results[2 * b]["out"].astype(np.float32)
        out[b] += res.results[2 * b + 1]["out"].astype(np.float32)

    # exact host-side correction for v/output biases (zero in this problem)
    if np.any(bv) or np.any(bo):
        out += bv.reshape(E) @ Wo + bo
    return out
